# revision 40
# baseline (speedup 1.0000x reference)
"""MoE model (router + top-2 of 8 experts, 3-layer MLP experts) on 8 trn2 cores.

Strategy: expert-parallel. The router (a tiny [4096,512]@[512,8] matmul +
softmax + top-2) runs on the host; tokens are gathered per expert on the host
and shipped to the core owning that expert. Each core runs the 3-layer MLP for
its expert on its token set in a transposed [feature, token] layout so every
layer is a plain lhsT.T @ rhs chain on the tensor engine with no on-device
transposes. The host scatters the per-expert outputs back and applies the
top-2 gate weights.

Perf notes:
- Matmuls run in float32r (fast fp32, 1 cycle/row for even moving dim >= 256).
  The PE rounds raw f32 inputs internally, so tensors are DMA'd straight into
  float32r SBUF tiles (verified bit-identical to an explicit cast on HW).
- Every DMA generates one descriptor per SBUF partition, so host arrays are
  pre-packed partition-major and each tensor moves in as few wide DMAs as
  possible (128 descriptors each), split only where needed to pipeline with
  compute. All DMAs go through the sync HWDGE queue (scalar-queue DMA
  crashed the device).
- The steady state is LDWEIGHTS-bound (~171 ns per 128x128 f32r weight
  load), so token chunks are equal-split (moving dim ~386 < 512) and the
  matmul count, 68 per chunk, is what matters; a few f32 warm-up matmuls
  run during the load phase so the HAM clock gate is at 2.4 GHz when real
  work starts.

Self-contained: hardcodes all shapes from the problem spec.
"""

import numpy as np

B = 4096
D = 512
H1 = 1024
H2 = 512
C = 50
E = 8
TOP_K = 2
P = 128
N_CORES = 8

K1 = D // P   # 4 k-subtiles for layer 1 / layer 3
M1 = H1 // P  # 8 output tiles for layer 1
K2 = H1 // P  # 8 k-subtiles for layer 2
M2 = H2 // P  # 4 output tiles for layer 2

# "f32r" (fast fp32, ~1 cyc/row, rel err ~2e-4) or "f32" (exact, 4 cyc/row).
MM_DTYPE = "f32r"

_program_cache: dict = {}


def _token_chunks(cap: int) -> list[tuple[int, int]]:
    """Split cap into equal-ish (start, size) chunks of at most 512 columns.

    Equal splitting keeps every chunk >= 256 whenever cap >= 512, so float32r
    matmuls stay at full rate. Sizes are even (f32r ISA requirement); cap must
    be even."""
    assert cap % 2 == 0
    parts = max(1, -(-cap // 512))
    half = cap // 2
    base, rem = divmod(half, parts)
    sizes = [2 * (base + 1)] * rem + [2 * base] * (parts - rem)
    out, start = [], 0
    for s in sizes:
        out.append((start, s))
        start += s
    return out


def _build_program(cap: int):
    import concourse.mybir as mybir
    import concourse.tile as tile
    from concourse import bacc

    f32 = mybir.dt.float32
    mm_dt = mybir.dt.float32r if MM_DTYPE == "f32r" else mybir.dt.float32

    nc = bacc.Bacc("TRN2", target_bir_lowering=False, debug=False)
    xT_d = nc.dram_tensor("xTp", [P, K1 * cap], mm_dt, kind="ExternalInput")
    w1_d = nc.dram_tensor("w1p", [P, M1 * K1 * P], mm_dt, kind="ExternalInput")
    w2_d = nc.dram_tensor("w2p", [P, M2 * K2 * P], mm_dt, kind="ExternalInput")
    w3_d = nc.dram_tensor("w3p", [P, K1 * C], mm_dt, kind="ExternalInput")
    b123_d = nc.dram_tensor("b123", [P, M1 + M2 + 1], f32, kind="ExternalInput")
    y_d = nc.dram_tensor("y", [C, cap], f32, kind="ExternalOutput")

    chunks = _token_chunks(cap)
    relu = mybir.ActivationFunctionType.Relu

    with tile.TileContext(nc) as tc:
        with (
            tc.tile_pool(name="sb", bufs=1) as sb,
            tc.tile_pool(name="ps", bufs=8, space="PSUM") as pspool,
        ):
            xcs = []
            for ci, (n0, nt) in enumerate(chunks):
                xc = sb.tile([P, K1, nt], mm_dt, tag=f"xt{ci}", name="xc")
                xcs.append(xc)
            w1s = sb.tile([P, M1, K1, P], mm_dt, tag="w1")
            w2s = sb.tile([P, M2, K2, P], mm_dt, tag="w2")
            w3s = sb.tile([P, K1, C], mm_dt, tag="w3")
            b123s = sb.tile([P, M1 + M2 + 1], f32, tag="b123")
            b1s = b123s[:, :M1]
            b2s = b123s[:, M1 : M1 + M2]
            b3s = b123s[:C, M1 + M2 : M1 + M2 + 1]
            h1s = sb.tile([P, K2, cap], mm_dt, tag="h1")
            h2s = sb.tile([P, M2, cap], mm_dt, tag="h2")
            ys = sb.tile([C, cap], f32, tag="y")
            zdummy = sb.tile([P, 256], f32, tag="zd")

            # PE prewarm: back-to-back dummy matmuls on zeroed SBUF run
            # while input DMAs stream, so the HAM clock gate is at full rate
            # (2.4 GHz) when the real matmuls start.
            nc.vector.memset(zdummy[:], 0)
            psd = pspool.tile([P, 512], f32, tag="ps", name="psd")
            for _ in range(7):
                nc.tensor.matmul(
                    psd[:, :128], zdummy[:, :P], zdummy[:, :128], start=True, stop=True
                )

            def xc_dma(ci, k_lo=0, k_hi=K1, eng=None):
                n0, nt = chunks[ci]
                (eng or nc.sync).dma_start(
                    xcs[ci][:, k_lo:k_hi],
                    xT_d[
                        :, K1 * n0 + k_lo * nt : K1 * n0 + k_hi * nt
                    ].rearrange("p (k n) -> p k n", k=k_hi - k_lo),
                )

            def w_dma(dst, src_d, m_lo, m_hi, ksub):
                nc.sync.dma_start(
                    dst[:, m_lo:m_hi],
                    src_d[:, m_lo * ksub * P : m_hi * ksub * P].rearrange(
                        "p (m k c) -> p m k c", m=m_hi - m_lo, k=ksub
                    ),
                )

            # DMA order tracks first-use: pieces sized so the PE never waits
            # long on the next piece.
            xc_dma(0)
            w_dma(w1s, w1_d, 0, 1, K1)
            w_dma(w1s, w1_d, 1, 2, K1)
            nc.gpsimd.dma_start(b123s[:], b123_d[:])
            w_dma(w1s, w1_d, 2, 4, K1)
            w_dma(w1s, w1_d, 4, 6, K1)
            w_dma(w1s, w1_d, 6, M1, K1)
            for ci in range(1, len(chunks)):
                xc_dma(ci, eng=nc.gpsimd)
            w_dma(w2s, w2_d, 0, 1, K2)
            w_dma(w2s, w2_d, 1, 2, K2)
            w_dma(w2s, w2_d, 2, 4, K2)
            nc.gpsimd.dma_start(w3s[:].rearrange("p k c -> p (k c)"), w3_d[:])

            # Layer-major compute order: by the time a layer-2 (or 3) phase
            # runs, its chunk's inputs drained phases ago, so the PE never
            # waits on an activation drain at a phase boundary.
            for ci, (n0, nt) in enumerate(chunks):
                # Layer 1: h1 = relu(W1.T @ xT + b1); drains alternate between
                # the scalar and vector engines so they keep pace with the PE.
                for m in range(M1):
                    ps = pspool.tile([P, 512], f32, tag="ps", name="ps")[:, :nt]
                    for k in range(K1):
                        nc.tensor.matmul(
                            ps,
                            w1s[:, m, k, :],
                            xcs[ci][:, k, :],
                            start=(k == 0),
                            stop=(k == K1 - 1),
                        )
                    if m % 2 == 0:
                        nc.scalar.activation(
                            h1s[:, m, n0 : n0 + nt], ps, relu, bias=b1s[:, m : m + 1]
                        )
                    else:
                        nc.vector.tensor_scalar(
                            h1s[:, m, n0 : n0 + nt],
                            ps,
                            b1s[:, m : m + 1],
                            0.0,
                            mybir.AluOpType.add,
                            mybir.AluOpType.max,
                        )
            for ci, (n0, nt) in enumerate(chunks):
                # Layer 2: h2 = relu(W2.T @ h1 + b2)
                for m in range(M2):
                    ps = pspool.tile([P, 512], f32, tag="ps", name="ps")[:, :nt]
                    for k in range(K2):
                        nc.tensor.matmul(
                            ps,
                            w2s[:, m, k, :],
                            h1s[:, k, n0 : n0 + nt],
                            start=(k == 0),
                            stop=(k == K2 - 1),
                        )
                    if m % 2 == 0:
                        nc.vector.tensor_scalar(
                            h2s[:, m, n0 : n0 + nt],
                            ps,
                            b2s[:, m : m + 1],
                            0.0,
                            mybir.AluOpType.add,
                            mybir.AluOpType.max,
                        )
                    else:
                        nc.scalar.activation(
                            h2s[:, m, n0 : n0 + nt], ps, relu, bias=b2s[:, m : m + 1]
                        )
            for ci, (n0, nt) in enumerate(chunks):
                # Layer 3: y = W3.T @ h2 + b3
                ps = pspool.tile([P, 512], f32, tag="ps", name="ps")[:C, :nt]
                for k in range(K1):
                    nc.tensor.matmul(
                        ps,
                        w3s[:, k, :],
                        h2s[:, k, n0 : n0 + nt],
                        start=(k == 0),
                        stop=(k == K1 - 1),
                    )
                nc.vector.tensor_scalar_add(ys[:, n0 : n0 + nt], ps, b3s[:, :1])
                nc.sync.dma_start(y_d[:, n0 : n0 + nt], ys[:, n0 : n0 + nt])

    nc.compile()

    # Prune the framework entry-block preamble: four const-tile memsets on
    # the slow-to-start GPSIMD engine plus an all-engine barrier waiting on
    # them (~4-5 us). This kernel never reads those consts (walrus flags
    # them as reader-less), and the barrier's semaphore protocol is
    # net-zero, so the exit-block barrier still starts from 0.
    const_read = any(
        str(getattr(arg, "memref", "")).startswith("const-")
        for fn in nc.m.functions
        for blk in fn.blocks
        for inst in blk.instructions
        for arg in (getattr(inst, "ins", None) or [])
    )
    if const_read:
        # something (e.g. a float-bias activation) reads a const tile; the
        # entry barrier is what orders its memset before use — keep it all.
        return nc
    entry = nc.m.functions[0].blocks[0]
    pruned = []
    for inst in entry.instructions:
        tn = type(inst).__name__
        if tn == "InstMemset" and inst.outs and str(
            getattr(inst.outs[0], "memref", "")
        ).startswith("const-"):
            continue
        if tn in ("InstDrain", "InstEventSemaphore"):
            si = getattr(inst, "sync_info", None)
            sems = [
                x.ant_name
                for x in ((si.on_wait or []) + (si.on_update or []))
            ] if si else []
            if tn == "InstDrain" and (
                not sems or all(s.startswith("barrier_") for s in sems)
            ):
                continue
            if tn == "InstEventSemaphore" and sems and all(
                s.startswith("barrier_") for s in sems
            ):
                continue
        pruned.append(inst)
    entry.instructions = pruned

    # Hoist the first few wait-free input DMAs into the entry block so they
    # issue right after TENSOR_LOAD instead of waiting for the sync engine
    # to branch into the main block (~2.5 us earlier).
    main_blk = nc.m.functions[0].blocks[1]
    hoist = []
    for inst in main_blk.instructions:
        if type(inst).__name__ != "InstDMACopy":
            continue
        si = getattr(inst, "sync_info", None)
        if si and si.on_wait:
            break
        hoist.append(inst)
        if len(hoist) == 3:
            break
    if hoist:
        main_blk.instructions = [
            i for i in main_blk.instructions if i not in hoist
        ]
        branch_at = next(
            idx
            for idx, i in enumerate(entry.instructions)
            if type(i).__name__ == "InstUnconditionalBranch"
        )
        entry.instructions = (
            entry.instructions[:branch_at]
            + hoist
            + entry.instructions[branch_at:]
        )
    return nc


def _get_program(cap: int):
    if cap not in _program_cache:
        _program_cache[cap] = _build_program(cap)
    return _program_cache[cap]


def _pack_biases(b1e, b2e, b3e):
    b = np.zeros((P, M1 + M2 + 1), dtype=np.float32)
    b[:, :M1] = b1e.reshape(M1, P).T
    b[:, M1 : M1 + M2] = b2e.reshape(M2, P).T
    b[:C, M1 + M2] = b3e
    return b


def _pack_inputs(x, W1, b1, W2, b2, W3, b3, tok_ids, counts, cap):
    chunks = _token_chunks(cap)
    in_maps = []
    for e in range(E):
        xe = np.zeros((cap, D), dtype=np.float32)
        xe[: counts[e]] = x[tok_ids[e]]
        xT = xe.T  # [D, cap]
        xTp = np.empty((P, K1 * cap), dtype=np.float32)
        for n0, nt in chunks:
            piece = xT[:, n0 : n0 + nt].reshape(K1, P, nt).transpose(1, 0, 2)
            xTp[:, K1 * n0 : K1 * (n0 + nt)] = piece.reshape(P, K1 * nt)
        in_maps.append(
            {
                "xTp": xTp,
                "w1p": np.ascontiguousarray(
                    W1[e].reshape(K1, P, M1, P).transpose(1, 2, 0, 3).reshape(P, -1)
                ),
                "w2p": np.ascontiguousarray(
                    W2[e].reshape(K2, P, M2, P).transpose(1, 2, 0, 3).reshape(P, -1)
                ),
                "w3p": np.ascontiguousarray(
                    W3[e].reshape(K1, P, C).transpose(1, 0, 2).reshape(P, -1)
                ),
                "b123": _pack_biases(b1[e], b2[e], b3[e]),
            }
        )
    return in_maps


def kernel(x, Wr, br, W1, b1, W2, b2, W3, b3, _run_opts=None):
    from concourse import bass_utils

    x = np.ascontiguousarray(np.asarray(x, dtype=np.float32))
    Wr = np.asarray(Wr, dtype=np.float32)
    br = np.asarray(br, dtype=np.float32)
    W1 = np.asarray(W1, dtype=np.float32)
    b1 = np.asarray(b1, dtype=np.float32)
    W2 = np.asarray(W2, dtype=np.float32)
    b2 = np.asarray(b2, dtype=np.float32)
    W3 = np.asarray(W3, dtype=np.float32)
    b3 = np.asarray(b3, dtype=np.float32)

    # ---- Router on host (tiny): probs = softmax(x @ Wr + br), top-2 ----
    logits = x @ Wr + br
    m = logits.max(axis=1, keepdims=True)
    ex = np.exp(logits - m)
    probs = ex / ex.sum(axis=1, keepdims=True)
    # stable argsort matches jax.lax.top_k tie-breaking (lowest index first)
    top2 = np.argsort(-probs, axis=1, kind="stable")[:, :TOP_K]

    tok_ids = []
    gates = []
    for e in range(E):
        te = np.nonzero((top2 == e).any(axis=1))[0]
        tok_ids.append(te)
        gates.append(probs[te, e])
    counts = [len(t) for t in tok_ids]
    cap = max(64, max(counts))
    cap += cap % 2

    nc = _get_program(cap)
    in_maps = _pack_inputs(x, W1, b1, W2, b2, W3, b3, tok_ids, counts, cap)

    run_opts = dict(_run_opts or {})
    res = bass_utils.run_bass_kernel_spmd(
        nc, in_maps, core_ids=list(range(N_CORES)), **run_opts
    )

    out = np.zeros((B, C), dtype=np.float32)
    for e in range(E):
        ye = res.results[e]["y"][:, : counts[e]].T  # [count, C]
        out[tok_ids[e]] += gates[e][:, None] * ye
    out *= 1.0 / TOP_K

    if _run_opts is not None:
        return (out, probs), res
    return out, probs


# revision 41
# speedup vs baseline: 1.0819x; 1.0819x over previous
"""MoE model (router + top-2 of 8 experts, 3-layer MLP experts) on 8 trn2 cores.

Strategy: expert-parallel. The router (a tiny [4096,512]@[512,8] matmul +
softmax + top-2) runs on the host; tokens are gathered per expert on the host
and shipped to the core owning that expert. Each core runs the 3-layer MLP for
its expert on its token set in a transposed [feature, token] layout so every
layer is a plain lhsT.T @ rhs chain on the tensor engine with no on-device
transposes. The host scatters the per-expert outputs back and applies the
top-2 gate weights.

Perf notes:
- Matmuls run in float32r (fast fp32, 1 cycle/row for even moving dim >= 256).
  The PE rounds raw f32 inputs internally, so tensors are DMA'd straight into
  float32r SBUF tiles (verified bit-identical to an explicit cast on HW).
- Every DMA generates one descriptor per SBUF partition, so host arrays are
  pre-packed partition-major and each tensor moves in as few wide DMAs as
  possible (128 descriptors each), split only where needed to pipeline with
  compute. All DMAs go through the sync HWDGE queue (scalar-queue DMA
  crashed the device).
- The steady state is LDWEIGHTS-bound (~171 ns per 128x128 f32r weight
  load), so token chunks are equal-split (moving dim ~386 < 512) and the
  matmul count, 68 per chunk, is what matters; a few f32 warm-up matmuls
  run during the load phase so the HAM clock gate is at 2.4 GHz when real
  work starts.

Self-contained: hardcodes all shapes from the problem spec.
"""

import numpy as np

B = 4096
D = 512
H1 = 1024
H2 = 512
C = 50
E = 8
TOP_K = 2
P = 128
N_CORES = 8

K1 = D // P   # 4 k-subtiles for layer 1 / layer 3
M1 = H1 // P  # 8 output tiles for layer 1
K2 = H1 // P  # 8 k-subtiles for layer 2
M2 = H2 // P  # 4 output tiles for layer 2

# "f32r" (fast fp32, ~1 cyc/row, rel err ~2e-4) or "f32" (exact, 4 cyc/row).
MM_DTYPE = "f32r"

_program_cache: dict = {}


def _token_chunks(cap: int) -> list[tuple[int, int]]:
    """Split cap into equal-ish (start, size) chunks of at most 512 columns.

    Equal splitting keeps every chunk >= 256 whenever cap >= 512, so float32r
    matmuls stay at full rate. Sizes are even (f32r ISA requirement); cap must
    be even."""
    assert cap % 2 == 0
    parts = max(1, -(-cap // 512))
    half = cap // 2
    base, rem = divmod(half, parts)
    sizes = [2 * (base + 1)] * rem + [2 * base] * (parts - rem)
    out, start = [], 0
    for s in sizes:
        out.append((start, s))
        start += s
    return out


def _build_program(cap: int):
    import concourse.mybir as mybir
    import concourse.tile as tile
    from concourse import bacc

    f32 = mybir.dt.float32
    mm_dt = mybir.dt.float32r if MM_DTYPE == "f32r" else mybir.dt.float32

    nc = bacc.Bacc("TRN2", target_bir_lowering=False, debug=False)
    xT_d = nc.dram_tensor("xTp", [P, K1 * cap], mm_dt, kind="ExternalInput")
    w1_d = nc.dram_tensor("w1p", [P, M1 * K1 * P], mm_dt, kind="ExternalInput")
    w2_d = nc.dram_tensor("w2p", [P, M2 * K2 * P], mm_dt, kind="ExternalInput")
    w3_d = nc.dram_tensor("w3p", [P, K1 * C], mm_dt, kind="ExternalInput")
    b123_d = nc.dram_tensor("b123", [P, M1 + M2 + 1], f32, kind="ExternalInput")
    y_d = nc.dram_tensor("y", [C, cap], f32, kind="ExternalOutput")

    chunks = _token_chunks(cap)
    relu = mybir.ActivationFunctionType.Relu

    with tile.TileContext(nc) as tc:
        with (
            tc.tile_pool(name="sb", bufs=1) as sb,
            tc.tile_pool(name="ps", bufs=8, space="PSUM") as pspool,
        ):
            xcs = []
            for ci, (n0, nt) in enumerate(chunks):
                xc = sb.tile([P, K1, nt], mm_dt, tag=f"xt{ci}", name="xc")
                xcs.append(xc)
            w1s = sb.tile([P, M1, K1, P], mm_dt, tag="w1")
            w2s = sb.tile([P, M2, K2, P], mm_dt, tag="w2")
            w3s = sb.tile([P, K1, C], mm_dt, tag="w3")
            b123s = sb.tile([P, M1 + M2 + 1], f32, tag="b123")
            b1s = b123s[:, :M1]
            b2s = b123s[:, M1 : M1 + M2]
            b3s = b123s[:C, M1 + M2 : M1 + M2 + 1]
            h1s = sb.tile([P, K2, cap], mm_dt, tag="h1")
            h2s = sb.tile([P, M2, cap], mm_dt, tag="h2")
            ys = sb.tile([C, cap], f32, tag="y")
            zdummy = sb.tile([P, 256], f32, tag="zd")

            # PE prewarm: back-to-back dummy matmuls on zeroed SBUF run
            # while input DMAs stream, so the HAM clock gate is at full rate
            # (2.4 GHz) when the real matmuls start.
            nc.vector.memset(zdummy[:], 0)
            psd = pspool.tile([P, 512], f32, tag="ps", name="psd")
            for _ in range(7):
                nc.tensor.matmul(
                    psd[:, :128], zdummy[:, :P], zdummy[:, :128], start=True, stop=True
                )

            def xc_dma(ci, k_lo=0, k_hi=K1):
                n0, nt = chunks[ci]
                nc.sync.dma_start(
                    xcs[ci][:, k_lo:k_hi],
                    xT_d[
                        :, K1 * n0 + k_lo * nt : K1 * n0 + k_hi * nt
                    ].rearrange("p (k n) -> p k n", k=k_hi - k_lo),
                )

            def w_dma(dst, src_d, m_lo, m_hi, ksub):
                nc.sync.dma_start(
                    dst[:, m_lo:m_hi],
                    src_d[:, m_lo * ksub * P : m_hi * ksub * P].rearrange(
                        "p (m k c) -> p m k c", m=m_hi - m_lo, k=ksub
                    ),
                )

            # DMA order tracks first-use: pieces sized so the PE never waits
            # long on the next piece.
            xc_dma(0)
            w_dma(w1s, w1_d, 0, 1, K1)
            w_dma(w1s, w1_d, 1, 2, K1)
            nc.sync.dma_start(b123s[:], b123_d[:])
            w_dma(w1s, w1_d, 2, 4, K1)
            w_dma(w1s, w1_d, 4, 6, K1)
            w_dma(w1s, w1_d, 6, M1, K1)
            for ci in range(1, len(chunks)):
                xc_dma(ci)
            w_dma(w2s, w2_d, 0, 1, K2)
            w_dma(w2s, w2_d, 1, 2, K2)
            w_dma(w2s, w2_d, 2, 4, K2)
            nc.sync.dma_start(w3s[:].rearrange("p k c -> p (k c)"), w3_d[:])

            # Layer-major compute order: by the time a layer-2 (or 3) phase
            # runs, its chunk's inputs drained phases ago, so the PE never
            # waits on an activation drain at a phase boundary.
            for ci, (n0, nt) in enumerate(chunks):
                # Layer 1: h1 = relu(W1.T @ xT + b1); drains alternate between
                # the scalar and vector engines so they keep pace with the PE.
                for m in range(M1):
                    ps = pspool.tile([P, 512], f32, tag="ps", name="ps")[:, :nt]
                    for k in range(K1):
                        nc.tensor.matmul(
                            ps,
                            w1s[:, m, k, :],
                            xcs[ci][:, k, :],
                            start=(k == 0),
                            stop=(k == K1 - 1),
                        )
                    if m % 2 == 0:
                        nc.scalar.activation(
                            h1s[:, m, n0 : n0 + nt], ps, relu, bias=b1s[:, m : m + 1]
                        )
                    else:
                        nc.vector.tensor_scalar(
                            h1s[:, m, n0 : n0 + nt],
                            ps,
                            b1s[:, m : m + 1],
                            0.0,
                            mybir.AluOpType.add,
                            mybir.AluOpType.max,
                        )
            for ci, (n0, nt) in enumerate(chunks):
                # Layer 2: h2 = relu(W2.T @ h1 + b2)
                for m in range(M2):
                    ps = pspool.tile([P, 512], f32, tag="ps", name="ps")[:, :nt]
                    for k in range(K2):
                        nc.tensor.matmul(
                            ps,
                            w2s[:, m, k, :],
                            h1s[:, k, n0 : n0 + nt],
                            start=(k == 0),
                            stop=(k == K2 - 1),
                        )
                    if m % 2 == 0:
                        nc.vector.tensor_scalar(
                            h2s[:, m, n0 : n0 + nt],
                            ps,
                            b2s[:, m : m + 1],
                            0.0,
                            mybir.AluOpType.add,
                            mybir.AluOpType.max,
                        )
                    else:
                        nc.scalar.activation(
                            h2s[:, m, n0 : n0 + nt], ps, relu, bias=b2s[:, m : m + 1]
                        )
            for ci, (n0, nt) in enumerate(chunks):
                # Layer 3: y = W3.T @ h2 + b3
                ps = pspool.tile([P, 512], f32, tag="ps", name="ps")[:C, :nt]
                for k in range(K1):
                    nc.tensor.matmul(
                        ps,
                        w3s[:, k, :],
                        h2s[:, k, n0 : n0 + nt],
                        start=(k == 0),
                        stop=(k == K1 - 1),
                    )
                nc.vector.tensor_scalar_add(ys[:, n0 : n0 + nt], ps, b3s[:, :1])
                nc.sync.dma_start(y_d[:, n0 : n0 + nt], ys[:, n0 : n0 + nt])

    nc.compile()

    # Prune the framework entry-block preamble: four const-tile memsets on
    # the slow-to-start GPSIMD engine plus an all-engine barrier waiting on
    # them (~4-5 us). This kernel never reads those consts (walrus flags
    # them as reader-less), and the barrier's semaphore protocol is
    # net-zero, so the exit-block barrier still starts from 0.
    const_read = any(
        str(getattr(arg, "memref", "")).startswith("const-")
        for fn in nc.m.functions
        for blk in fn.blocks
        for inst in blk.instructions
        for arg in (getattr(inst, "ins", None) or [])
    )
    if const_read:
        # something (e.g. a float-bias activation) reads a const tile; the
        # entry barrier is what orders its memset before use — keep it all.
        return nc
    entry = nc.m.functions[0].blocks[0]
    pruned = []
    for inst in entry.instructions:
        tn = type(inst).__name__
        if tn == "InstMemset" and inst.outs and str(
            getattr(inst.outs[0], "memref", "")
        ).startswith("const-"):
            continue
        if tn in ("InstDrain", "InstEventSemaphore"):
            si = getattr(inst, "sync_info", None)
            sems = [
                x.ant_name
                for x in ((si.on_wait or []) + (si.on_update or []))
            ] if si else []
            if tn == "InstDrain" and (
                not sems or all(s.startswith("barrier_") for s in sems)
            ):
                continue
            if tn == "InstEventSemaphore" and sems and all(
                s.startswith("barrier_") for s in sems
            ):
                continue
        pruned.append(inst)
    entry.instructions = pruned

    # Hoist the first few wait-free input DMAs into the entry block so they
    # issue right after TENSOR_LOAD instead of waiting for the sync engine
    # to branch into the main block (~2.5 us earlier).
    main_blk = nc.m.functions[0].blocks[1]
    hoist = []
    for inst in main_blk.instructions:
        if type(inst).__name__ != "InstDMACopy":
            continue
        si = getattr(inst, "sync_info", None)
        if si and si.on_wait:
            break
        hoist.append(inst)
        if len(hoist) == 3:
            break
    if hoist:
        main_blk.instructions = [
            i for i in main_blk.instructions if i not in hoist
        ]
        branch_at = next(
            idx
            for idx, i in enumerate(entry.instructions)
            if type(i).__name__ == "InstUnconditionalBranch"
        )
        entry.instructions = (
            entry.instructions[:branch_at]
            + hoist
            + entry.instructions[branch_at:]
        )
    return nc


def _get_program(cap: int):
    if cap not in _program_cache:
        _program_cache[cap] = _build_program(cap)
    return _program_cache[cap]


def _pack_biases(b1e, b2e, b3e):
    b = np.zeros((P, M1 + M2 + 1), dtype=np.float32)
    b[:, :M1] = b1e.reshape(M1, P).T
    b[:, M1 : M1 + M2] = b2e.reshape(M2, P).T
    b[:C, M1 + M2] = b3e
    return b


def _pack_inputs(x, W1, b1, W2, b2, W3, b3, tok_ids, counts, cap):
    chunks = _token_chunks(cap)
    in_maps = []
    for e in range(E):
        xe = np.zeros((cap, D), dtype=np.float32)
        xe[: counts[e]] = x[tok_ids[e]]
        xT = xe.T  # [D, cap]
        xTp = np.empty((P, K1 * cap), dtype=np.float32)
        for n0, nt in chunks:
            piece = xT[:, n0 : n0 + nt].reshape(K1, P, nt).transpose(1, 0, 2)
            xTp[:, K1 * n0 : K1 * (n0 + nt)] = piece.reshape(P, K1 * nt)
        in_maps.append(
            {
                "xTp": xTp,
                "w1p": np.ascontiguousarray(
                    W1[e].reshape(K1, P, M1, P).transpose(1, 2, 0, 3).reshape(P, -1)
                ),
                "w2p": np.ascontiguousarray(
                    W2[e].reshape(K2, P, M2, P).transpose(1, 2, 0, 3).reshape(P, -1)
                ),
                "w3p": np.ascontiguousarray(
                    W3[e].reshape(K1, P, C).transpose(1, 0, 2).reshape(P, -1)
                ),
                "b123": _pack_biases(b1[e], b2[e], b3[e]),
            }
        )
    return in_maps


def kernel(x, Wr, br, W1, b1, W2, b2, W3, b3, _run_opts=None):
    from concourse import bass_utils

    x = np.ascontiguousarray(np.asarray(x, dtype=np.float32))
    Wr = np.asarray(Wr, dtype=np.float32)
    br = np.asarray(br, dtype=np.float32)
    W1 = np.asarray(W1, dtype=np.float32)
    b1 = np.asarray(b1, dtype=np.float32)
    W2 = np.asarray(W2, dtype=np.float32)
    b2 = np.asarray(b2, dtype=np.float32)
    W3 = np.asarray(W3, dtype=np.float32)
    b3 = np.asarray(b3, dtype=np.float32)

    # ---- Router on host (tiny): probs = softmax(x @ Wr + br), top-2 ----
    logits = x @ Wr + br
    m = logits.max(axis=1, keepdims=True)
    ex = np.exp(logits - m)
    probs = ex / ex.sum(axis=1, keepdims=True)
    # stable argsort matches jax.lax.top_k tie-breaking (lowest index first)
    top2 = np.argsort(-probs, axis=1, kind="stable")[:, :TOP_K]

    tok_ids = []
    gates = []
    for e in range(E):
        te = np.nonzero((top2 == e).any(axis=1))[0]
        tok_ids.append(te)
        gates.append(probs[te, e])
    counts = [len(t) for t in tok_ids]
    cap = max(64, max(counts))
    cap += cap % 2

    nc = _get_program(cap)
    in_maps = _pack_inputs(x, W1, b1, W2, b2, W3, b3, tok_ids, counts, cap)

    run_opts = dict(_run_opts or {})
    res = bass_utils.run_bass_kernel_spmd(
        nc, in_maps, core_ids=list(range(N_CORES)), **run_opts
    )

    out = np.zeros((B, C), dtype=np.float32)
    for e in range(E):
        ye = res.results[e]["y"][:, : counts[e]].T  # [count, C]
        out[tok_ids[e]] += gates[e][:, None] * ye
    out *= 1.0 / TOP_K

    if _run_opts is not None:
        return (out, probs), res
    return out, probs


# revision 42
# speedup vs baseline: 1.1081x; 1.0242x over previous
"""MoE model (router + top-2 of 8 experts, 3-layer MLP experts) on 8 trn2 cores.

Strategy: expert-parallel. The router (a tiny [4096,512]@[512,8] matmul +
softmax + top-2) runs on the host; tokens are gathered per expert on the host
and shipped to the core owning that expert. Each core runs the 3-layer MLP for
its expert on its token set in a transposed [feature, token] layout so every
layer is a plain lhsT.T @ rhs chain on the tensor engine with no on-device
transposes. The host scatters the per-expert outputs back and applies the
top-2 gate weights.

Perf notes:
- Matmuls run in float32r (fast fp32, 1 cycle/row for even moving dim >= 256).
  The PE rounds raw f32 inputs internally, so tensors are DMA'd straight into
  float32r SBUF tiles (verified bit-identical to an explicit cast on HW).
- Every DMA generates one descriptor per SBUF partition, so host arrays are
  pre-packed partition-major and each tensor moves in as few wide DMAs as
  possible (128 descriptors each), split only where needed to pipeline with
  compute. All DMAs go through the sync HWDGE queue (scalar-queue DMA
  crashed the device).
- The steady state is LDWEIGHTS-bound (~171 ns per 128x128 f32r weight
  load), so token chunks are equal-split (moving dim ~386 < 512) and the
  matmul count, 68 per chunk, is what matters; a few f32 warm-up matmuls
  run during the load phase so the HAM clock gate is at 2.4 GHz when real
  work starts.

Self-contained: hardcodes all shapes from the problem spec.
"""

import numpy as np

B = 4096
D = 512
H1 = 1024
H2 = 512
C = 50
E = 8
TOP_K = 2
P = 128
N_CORES = 8

K1 = D // P   # 4 k-subtiles for layer 1 / layer 3
M1 = H1 // P  # 8 output tiles for layer 1
K2 = H1 // P  # 8 k-subtiles for layer 2
M2 = H2 // P  # 4 output tiles for layer 2

# "f32r" (fast fp32, ~1 cyc/row, rel err ~2e-4) or "f32" (exact, 4 cyc/row).
MM_DTYPE = "f32r"

_program_cache: dict = {}


def _token_chunks(cap: int) -> list[tuple[int, int]]:
    """Split cap into equal-ish (start, size) chunks of at most 512 columns.

    Equal splitting keeps every chunk >= 256 whenever cap >= 512, so float32r
    matmuls stay at full rate. Sizes are even (f32r ISA requirement); cap must
    be even."""
    assert cap % 2 == 0
    parts = max(1, -(-cap // 512))
    half = cap // 2
    base, rem = divmod(half, parts)
    sizes = [2 * (base + 1)] * rem + [2 * base] * (parts - rem)
    out, start = [], 0
    for s in sizes:
        out.append((start, s))
        start += s
    return out


def _build_program(cap: int):
    import concourse.mybir as mybir
    import concourse.tile as tile
    from concourse import bacc

    f32 = mybir.dt.float32
    mm_dt = mybir.dt.float32r if MM_DTYPE == "f32r" else mybir.dt.float32

    nc = bacc.Bacc("TRN2", target_bir_lowering=False, debug=False)
    xT_d = nc.dram_tensor("xTp", [P, K1 * cap], mm_dt, kind="ExternalInput")
    w1_d = nc.dram_tensor("w1p", [P, M1 * K1 * P], mm_dt, kind="ExternalInput")
    w2_d = nc.dram_tensor("w2p", [P, M2 * K2 * P], mm_dt, kind="ExternalInput")
    w3_d = nc.dram_tensor("w3p", [P, K1 * C], mm_dt, kind="ExternalInput")
    b123_d = nc.dram_tensor("b123", [P, M1 + M2 + 1], f32, kind="ExternalInput")
    y_d = nc.dram_tensor("y", [C, cap], f32, kind="ExternalOutput")

    chunks = _token_chunks(cap)
    relu = mybir.ActivationFunctionType.Relu

    with tile.TileContext(nc) as tc:
        with (
            tc.tile_pool(name="sb", bufs=1) as sb,
            tc.tile_pool(name="ps", bufs=8, space="PSUM") as pspool,
        ):
            xcs = []
            for ci, (n0, nt) in enumerate(chunks):
                xc = sb.tile([P, K1, nt], mm_dt, tag=f"xt{ci}", name="xc")
                xcs.append(xc)
            w1s = sb.tile([P, M1, K1, P], mm_dt, tag="w1")
            w2s = sb.tile([P, M2, K2, P], mm_dt, tag="w2")
            w3s = sb.tile([P, K1, C], mm_dt, tag="w3")
            b123s = sb.tile([P, M1 + M2 + 1], f32, tag="b123")
            b1s = b123s[:, :M1]
            b2s = b123s[:, M1 : M1 + M2]
            b3s = b123s[:C, M1 + M2 : M1 + M2 + 1]
            h1s = sb.tile([P, K2, cap], mm_dt, tag="h1")
            h2s = sb.tile([P, M2, cap], mm_dt, tag="h2")
            ys = sb.tile([C, cap], f32, tag="y")
            zdummy = sb.tile([P, 256], f32, tag="zd")

            # PE prewarm: back-to-back dummy matmuls on zeroed SBUF run
            # while input DMAs stream, so the HAM clock gate is at full rate
            # (2.4 GHz) when the real matmuls start.
            nc.vector.memset(zdummy[:], 0)
            psd = pspool.tile([P, 512], f32, tag="ps", name="psd")
            for _ in range(10):
                nc.tensor.matmul(
                    psd[:, :128], zdummy[:, :P], zdummy[:, :128], start=True, stop=True
                )

            def xc_dma(ci, k_lo=0, k_hi=K1):
                n0, nt = chunks[ci]
                nc.sync.dma_start(
                    xcs[ci][:, k_lo:k_hi],
                    xT_d[
                        :, K1 * n0 + k_lo * nt : K1 * n0 + k_hi * nt
                    ].rearrange("p (k n) -> p k n", k=k_hi - k_lo),
                )

            def w_dma(dst, src_d, m_lo, m_hi, ksub):
                nc.sync.dma_start(
                    dst[:, m_lo:m_hi],
                    src_d[:, m_lo * ksub * P : m_hi * ksub * P].rearrange(
                        "p (m k c) -> p m k c", m=m_hi - m_lo, k=ksub
                    ),
                )

            # DMA order tracks first-use: pieces sized so the PE never waits
            # long on the next piece.
            xc_dma(0)
            w_dma(w1s, w1_d, 0, 1, K1)
            w_dma(w1s, w1_d, 1, 2, K1)
            nc.sync.dma_start(b123s[:], b123_d[:])
            w_dma(w1s, w1_d, 2, 4, K1)
            w_dma(w1s, w1_d, 4, 6, K1)
            w_dma(w1s, w1_d, 6, M1, K1)
            for ci in range(1, len(chunks)):
                xc_dma(ci)
            w_dma(w2s, w2_d, 0, 1, K2)
            w_dma(w2s, w2_d, 1, 2, K2)
            w_dma(w2s, w2_d, 2, 4, K2)
            nc.sync.dma_start(w3s[:].rearrange("p k c -> p (k c)"), w3_d[:])

            # Layer-major compute order: by the time a layer-2 (or 3) phase
            # runs, its chunk's inputs drained phases ago, so the PE never
            # waits on an activation drain at a phase boundary.
            for ci, (n0, nt) in enumerate(chunks):
                # Layer 1: h1 = relu(W1.T @ xT + b1); drains alternate between
                # the scalar and vector engines so they keep pace with the PE.
                for m in range(M1):
                    ps = pspool.tile([P, 512], f32, tag="ps", name="ps")[:, :nt]
                    for k in range(K1):
                        nc.tensor.matmul(
                            ps,
                            w1s[:, m, k, :],
                            xcs[ci][:, k, :],
                            start=(k == 0),
                            stop=(k == K1 - 1),
                        )
                    if m % 2 == 0:
                        nc.scalar.activation(
                            h1s[:, m, n0 : n0 + nt], ps, relu, bias=b1s[:, m : m + 1]
                        )
                    else:
                        nc.vector.tensor_scalar(
                            h1s[:, m, n0 : n0 + nt],
                            ps,
                            b1s[:, m : m + 1],
                            0.0,
                            mybir.AluOpType.add,
                            mybir.AluOpType.max,
                        )
            for ci, (n0, nt) in enumerate(chunks):
                # Layer 2: h2 = relu(W2.T @ h1 + b2)
                for m in range(M2):
                    ps = pspool.tile([P, 512], f32, tag="ps", name="ps")[:, :nt]
                    for k in range(K2):
                        nc.tensor.matmul(
                            ps,
                            w2s[:, m, k, :],
                            h1s[:, k, n0 : n0 + nt],
                            start=(k == 0),
                            stop=(k == K2 - 1),
                        )
                    if m % 2 == 0:
                        nc.vector.tensor_scalar(
                            h2s[:, m, n0 : n0 + nt],
                            ps,
                            b2s[:, m : m + 1],
                            0.0,
                            mybir.AluOpType.add,
                            mybir.AluOpType.max,
                        )
                    else:
                        nc.scalar.activation(
                            h2s[:, m, n0 : n0 + nt], ps, relu, bias=b2s[:, m : m + 1]
                        )
            for ci, (n0, nt) in enumerate(chunks):
                # Layer 3: y = W3.T @ h2 + b3
                ps = pspool.tile([P, 512], f32, tag="ps", name="ps")[:C, :nt]
                for k in range(K1):
                    nc.tensor.matmul(
                        ps,
                        w3s[:, k, :],
                        h2s[:, k, n0 : n0 + nt],
                        start=(k == 0),
                        stop=(k == K1 - 1),
                    )
                nc.vector.tensor_scalar_add(ys[:, n0 : n0 + nt], ps, b3s[:, :1])
                nc.sync.dma_start(y_d[:, n0 : n0 + nt], ys[:, n0 : n0 + nt])

    nc.compile()

    # Prune the framework entry-block preamble: four const-tile memsets on
    # the slow-to-start GPSIMD engine plus an all-engine barrier waiting on
    # them (~4-5 us). This kernel never reads those consts (walrus flags
    # them as reader-less), and the barrier's semaphore protocol is
    # net-zero, so the exit-block barrier still starts from 0.
    const_read = any(
        str(getattr(arg, "memref", "")).startswith("const-")
        for fn in nc.m.functions
        for blk in fn.blocks
        for inst in blk.instructions
        for arg in (getattr(inst, "ins", None) or [])
    )
    if const_read:
        # something (e.g. a float-bias activation) reads a const tile; the
        # entry barrier is what orders its memset before use — keep it all.
        return nc
    entry = nc.m.functions[0].blocks[0]
    pruned = []
    for inst in entry.instructions:
        tn = type(inst).__name__
        if tn == "InstMemset" and inst.outs and str(
            getattr(inst.outs[0], "memref", "")
        ).startswith("const-"):
            continue
        if tn in ("InstDrain", "InstEventSemaphore"):
            si = getattr(inst, "sync_info", None)
            sems = [
                x.ant_name
                for x in ((si.on_wait or []) + (si.on_update or []))
            ] if si else []
            if tn == "InstDrain" and (
                not sems or all(s.startswith("barrier_") for s in sems)
            ):
                continue
            if tn == "InstEventSemaphore" and sems and all(
                s.startswith("barrier_") for s in sems
            ):
                continue
        pruned.append(inst)
    entry.instructions = pruned

    # Hoist the first few wait-free input DMAs into the entry block so they
    # issue right after TENSOR_LOAD instead of waiting for the sync engine
    # to branch into the main block (~2.5 us earlier).
    main_blk = nc.m.functions[0].blocks[1]
    hoist = []
    for inst in main_blk.instructions:
        if type(inst).__name__ != "InstDMACopy":
            continue
        si = getattr(inst, "sync_info", None)
        if si and si.on_wait:
            break
        hoist.append(inst)
        if len(hoist) == 3:
            break
    if hoist:
        main_blk.instructions = [
            i for i in main_blk.instructions if i not in hoist
        ]
        branch_at = next(
            idx
            for idx, i in enumerate(entry.instructions)
            if type(i).__name__ == "InstUnconditionalBranch"
        )
        entry.instructions = (
            entry.instructions[:branch_at]
            + hoist
            + entry.instructions[branch_at:]
        )
    return nc


def _get_program(cap: int):
    if cap not in _program_cache:
        _program_cache[cap] = _build_program(cap)
    return _program_cache[cap]


def _pack_biases(b1e, b2e, b3e):
    b = np.zeros((P, M1 + M2 + 1), dtype=np.float32)
    b[:, :M1] = b1e.reshape(M1, P).T
    b[:, M1 : M1 + M2] = b2e.reshape(M2, P).T
    b[:C, M1 + M2] = b3e
    return b


def _pack_inputs(x, W1, b1, W2, b2, W3, b3, tok_ids, counts, cap):
    chunks = _token_chunks(cap)
    in_maps = []
    for e in range(E):
        xe = np.zeros((cap, D), dtype=np.float32)
        xe[: counts[e]] = x[tok_ids[e]]
        xT = xe.T  # [D, cap]
        xTp = np.empty((P, K1 * cap), dtype=np.float32)
        for n0, nt in chunks:
            piece = xT[:, n0 : n0 + nt].reshape(K1, P, nt).transpose(1, 0, 2)
            xTp[:, K1 * n0 : K1 * (n0 + nt)] = piece.reshape(P, K1 * nt)
        in_maps.append(
            {
                "xTp": xTp,
                "w1p": np.ascontiguousarray(
                    W1[e].reshape(K1, P, M1, P).transpose(1, 2, 0, 3).reshape(P, -1)
                ),
                "w2p": np.ascontiguousarray(
                    W2[e].reshape(K2, P, M2, P).transpose(1, 2, 0, 3).reshape(P, -1)
                ),
                "w3p": np.ascontiguousarray(
                    W3[e].reshape(K1, P, C).transpose(1, 0, 2).reshape(P, -1)
                ),
                "b123": _pack_biases(b1[e], b2[e], b3[e]),
            }
        )
    return in_maps


def kernel(x, Wr, br, W1, b1, W2, b2, W3, b3, _run_opts=None):
    from concourse import bass_utils

    x = np.ascontiguousarray(np.asarray(x, dtype=np.float32))
    Wr = np.asarray(Wr, dtype=np.float32)
    br = np.asarray(br, dtype=np.float32)
    W1 = np.asarray(W1, dtype=np.float32)
    b1 = np.asarray(b1, dtype=np.float32)
    W2 = np.asarray(W2, dtype=np.float32)
    b2 = np.asarray(b2, dtype=np.float32)
    W3 = np.asarray(W3, dtype=np.float32)
    b3 = np.asarray(b3, dtype=np.float32)

    # ---- Router on host (tiny): probs = softmax(x @ Wr + br), top-2 ----
    logits = x @ Wr + br
    m = logits.max(axis=1, keepdims=True)
    ex = np.exp(logits - m)
    probs = ex / ex.sum(axis=1, keepdims=True)
    # stable argsort matches jax.lax.top_k tie-breaking (lowest index first)
    top2 = np.argsort(-probs, axis=1, kind="stable")[:, :TOP_K]

    tok_ids = []
    gates = []
    for e in range(E):
        te = np.nonzero((top2 == e).any(axis=1))[0]
        tok_ids.append(te)
        gates.append(probs[te, e])
    counts = [len(t) for t in tok_ids]
    cap = max(64, max(counts))
    cap += cap % 2

    nc = _get_program(cap)
    in_maps = _pack_inputs(x, W1, b1, W2, b2, W3, b3, tok_ids, counts, cap)

    run_opts = dict(_run_opts or {})
    res = bass_utils.run_bass_kernel_spmd(
        nc, in_maps, core_ids=list(range(N_CORES)), **run_opts
    )

    out = np.zeros((B, C), dtype=np.float32)
    for e in range(E):
        ye = res.results[e]["y"][:, : counts[e]].T  # [count, C]
        out[tok_ids[e]] += gates[e][:, None] * ye
    out *= 1.0 / TOP_K

    if _run_opts is not None:
        return (out, probs), res
    return out, probs
